# revision 1
# baseline (speedup 1.0000x reference)
"""Trainium2 kernel for nn_MLADecoderLayer (MLA attention + top-1 MoE).

Self-contained: shards the full inputs over 8 NeuronCores in two SPMD
launches (attention+router batch/seq-sharded, then MoE expert/hidden-
sharded with host all-to-all dispatch between them) and reassembles the
full output.
"""
import sys
if '/opt/trn_rl_repo' not in sys.path:
    sys.path.insert(0, '/opt/trn_rl_repo')

"""MLA decoder layer (attention + top-1 routed MoE + shared expert) on 8 trn2
NeuronCores, two SPMD launches.

Launch 1 (attention+router): 8 cores = (batch 2) x (seq-block 4); each core
computes 256 q rows against the full kv of its batch.  All big tensors are
kept feature-major ("transposed") so no large on-device transposes are
needed; rope is folded into host-prepared swapped weights + cos/sin tables;
the softmax denominator rides as an appended ones-column on V.

Launch 2 (MoE): 8 cores = (4 token chunks) x (2 hidden halves).  Chunks are
assigned to experts based on the measured routing (host all-to-all dispatch);
the shared expert occupies the first 512 hidden columns of every chunk so
each gathered token gets shared+routed in one pass.  Host sums the two
hidden-half partials and scatters back.
"""
import contextlib
import numpy as np
import ml_dtypes

import bass_rust
import concourse.bass as bass
import concourse.mybir as mybir
import concourse.tile as tile
from concourse.masks import make_identity
from concourse.vector_clock import ScopedClock

FP32 = mybir.dt.float32
BF16 = mybir.dt.bfloat16
AF = mybir.ActivationFunctionType
ALU = mybir.AluOpType

SEQ, BATCH, D, NH = 1024, 2, 1024, 16
ROPE = NOPE = 32
VD = 64
QL = 20
NEXP, HID = 4, 1024
SB = SEQ // 4
NT = 8
nbf = ml_dtypes.bfloat16


# --------------------------------------------------------------- wait split
MAX_WAITS = 1


def split_multi_waits(nc):
    """walrus in this container rejects instructions carrying more than one
    sync wait; hoist excess waits onto single-wait NOPs inserted before."""
    for bb in nc.main_func.blocks:
        il = bb.instructions
        i = 0
        while i < len(il):
            ins = il[i]
            si = ins.sync_info
            if si is not None and si.on_wait and len(si.on_wait) > MAX_WAITS:
                waits = list(si.on_wait)
                ins.sync_info = bass_rust.SyncInfo(
                    on_wait=waits[-MAX_WAITS:],
                    on_update=list(si.on_update) if si.on_update else [])
                rest = waits[:-MAX_WAITS]
                pre = []
                for j in range(0, len(rest), MAX_WAITS):
                    n = mybir.InstNoOp(name=f"I-waitsplit-{nc.next_id()}",
                                       ins=[], outs=[])
                    n.engine = ins.engine
                    n.sync_info = bass_rust.SyncInfo(
                        on_wait=rest[j:j + MAX_WAITS], on_update=[])
                    pre.append(n)
                for j, n in enumerate(pre):
                    il.insert(i + j, n)
                i += len(pre)
            i += 1


def bcast_row(nc, pspool, ones1, dst, src_row, tag="bc"):
    """Broadcast an SBUF row AP [1, N] to dst [P, N]: ones-matmul into PSUM
    (out[p, n] = 1 * row[n]) then copy back to SBUF."""
    P = dst.shape[0]
    N = dst.shape[-1]
    c0 = 0
    while c0 < N:
        cw = min(512, N - c0)
        ps = pspool.tile([P, cw], FP32, tag=tag)
        nc.tensor.matmul(ps, ones1[0:1, 0:P], src_row[:, c0:c0 + cw],
                         start=True, stop=True)
        nc.vector.tensor_copy(dst[:, c0:c0 + cw], ps)
        c0 += cw


# ================================================================= LAUNCH 1
def build_attn():
    nc = bass.Bass()

    def inp(name, shape, dt=BF16):
        return nc.dram_tensor(name, shape, dt, kind="ExternalInput")

    xT = inp("xT", [D, SEQ])            # x[:,bi,:]^T (per batch)
    xRT = inp("xRT", [D, SB])           # q-row column slice of xT (per core)
    wkv_cat = inp("wkv_cat", [D, 96])   # [ckv 20 | pad | kpe @32 | kpe_sw @64]
    wq_a = inp("wq_a", [D, QL])
    wq_cat = inp("wq_cat", [QL, 2048])  # [wq_b*scale 1024 | pe_sw padded 1024]
    wkv_nope = inp("wkv_nope", [QL, 1024])  # kcat-padded (zeros at pe cols)
    wkv_v = inp("wkv_v", [QL, 1024])
    wo = inp("wo", [D, D])
    gate_wT = inp("gate_wT", [D, NEXP], FP32)
    ck = inp("ck", [128, NT * SEQ])     # kcat-aligned rope tables
    sk = inp("sk", [128, NT * SEQ])
    cq = inp("cq", [128, NT * SB])      # qcat-aligned, per-core q-row slices
    sq = inp("sq", [128, NT * SB])
    sel = inp("sel", [16, NT * 128], FP32)  # head-select for denom broadcast

    yT_out = nc.dram_tensor("yT_out", [D, SB], FP32, kind="ExternalOutput")
    comb_out = nc.dram_tensor("comb_out", [SB, NEXP], FP32, kind="ExternalOutput")

    with tile.TileContext(nc) as tc, contextlib.ExitStack() as ctx:
        const = ctx.enter_context(tc.tile_pool(name="const", bufs=1))
        single = ctx.enter_context(tc.tile_pool(name="single", bufs=1))
        work = ctx.enter_context(tc.tile_pool(name="work", bufs=2))
        ppool = ctx.enter_context(tc.tile_pool(name="ppool", bufs=3))
        ps_mm = ctx.enter_context(tc.tile_pool(name="ps_mm", bufs=2, space="PSUM"))
        ps_sc = ctx.enter_context(tc.tile_pool(name="ps_sc", bufs=2, space="PSUM"))
        ps_att = ctx.enter_context(tc.tile_pool(name="ps_att", bufs=2, space="PSUM"))

        def load_tiled(dram, cols, dt=BF16, n=NT):
            t = const.tile([128, n, cols], dt, tag="ld_" + dram.name)
            for i in range(n):
                nc.sync.dma_start(out=t[:, i, :], in_=dram[i * 128:(i + 1) * 128, :])
            return t

        xT_b = load_tiled(xT, SEQ)
        xRT_b = load_tiled(xRT, SB)
        wkvc_b = load_tiled(wkv_cat, 96)
        wqa_b = load_tiled(wq_a, QL)
        wo_b = load_tiled(wo, D)
        gw_b = load_tiled(gate_wT, NEXP, FP32)
        def load_flat(dram, cols, tag):
            t = const.tile([128, NT, cols], BF16, tag=tag)
            nc.sync.dma_start(out=t, in_=dram[:, :])
            return t

        ck_b = load_flat(ck, SEQ, "ck")
        sk_b = load_flat(sk, SEQ, "sk")
        cq_b = load_flat(cq, SB, "cq")
        sq_b = load_flat(sq, SB, "sq")
        sel_b = const.tile([16, NT, 128], FP32, tag="sel")
        nc.sync.dma_start(out=sel_b, in_=sel[:, :])
        wqc_b = const.tile([QL, 2048], BF16)
        nc.sync.dma_start(out=wqc_b, in_=wq_cat[:, :])
        wkvn_b = const.tile([QL, 1024], BF16)
        nc.sync.dma_start(out=wkvn_b, in_=wkv_nope[:, :])
        wkvv_b = const.tile([QL, 1024], BF16)
        nc.sync.dma_start(out=wkvv_b, in_=wkv_v[:, :])
        ones20 = const.tile([QL, 1], BF16)
        nc.vector.memset(ones20, 1.0)
        ones1 = const.tile([1, 128], FP32)
        nc.vector.memset(ones1, 1.0)
        ident = const.tile([128, 128], FP32)
        make_identity(nc, ident)
        eps128 = const.tile([128, 1], FP32)
        nc.vector.memset(eps128, 1e-6)

        # big persistent SBUF buffers (per-tile lists for fine-grained deps)
        kcat = [const.tile([128, SEQ], BF16, tag=f"kcat{i}") for i in range(NT)]
        vbuf = [const.tile([128, NH, 65], BF16, tag=f"vbuf{i}") for i in range(NT)]
        qcat = [const.tile([128, SB], BF16, tag=f"qcat{i}") for i in range(NT)]
        attn_b = [const.tile([128, SB], BF16, tag=f"attb{i}") for i in range(NT)]
        attn_n = [const.tile([128, SB], BF16, tag=f"attn{i}") for i in range(NT)]
        yT_sb = [const.tile([128, SB], FP32, tag=f"yt{i}") for i in range(NT)]

        for i in range(NT):
            nc.vector.memset(vbuf[i][:, :, 64:65], 1.0)

        # ---------- stage A: x projections  proj^T [96, SEQ] (one 2-bank tile)
        proj = ps_mm.tile([96, 2, 512], FP32, tag="mm")
        for nh in range(2):
            for kt in range(NT):
                nc.tensor.matmul(proj[:, nh, :], wkvc_b[:, kt, :],
                                 xT_b[:, kt, nh * 512:(nh + 1) * 512],
                                 start=(kt == 0), stop=(kt == NT - 1))

        # rms scale over ckv rows (0..19) per seq column
        sq20 = single.tile([QL, SEQ], BF16, tag="sq20")
        nc.scalar.activation(sq20, proj[:QL, :, :], AF.Square)
        ms1 = ps_sc.tile([1, 512], FP32, tag="sc")
        ms2 = ps_sc.tile([1, 512], FP32, tag="sc")
        nc.tensor.matmul(ms1, ones20, sq20[:, 0:512], start=True, stop=True)
        nc.tensor.matmul(ms2, ones20, sq20[:, 512:1024], start=True, stop=True)
        sd = single.tile([1, SEQ], FP32, tag="sd")
        nc.scalar.activation(sd[:, 0:512], ms1, AF.Sqrt, bias=eps128[0:1, :], scale=1.0 / QL)
        nc.scalar.activation(sd[:, 512:1024], ms2, AF.Sqrt, bias=eps128[0:1, :], scale=1.0 / QL)
        rs = single.tile([1, SEQ], FP32, tag="rs")
        nc.vector.reciprocal(rs, sd)
        rs_b = single.tile([QL, SEQ], FP32, tag="rsb")
        bcast_row(nc, ps_sc, ones1, rs_b, rs[0:1, :], tag="sc")
        ckvn = single.tile([QL, SEQ], BF16, tag="ckvn")
        nc.vector.tensor_mul(ckvn, proj[:QL, :, :], rs_b)

        # k_pe and swapped variant replicated to all 4 32-row groups
        kpe_rep = single.tile([128, SEQ], BF16, tag="kpe")
        kpe2_rep = single.tile([128, SEQ], BF16, tag="kpe2")
        for g in range(4):
            gs = slice(g * 32, (g + 1) * 32)
            nc.vector.tensor_copy(kpe_rep[gs, :], proj[32:64, :, :])
            nc.vector.tensor_copy(kpe2_rep[gs, :], proj[64:96, :, :])

        # kcat: rope combine (full width) + kcat-padded k_nope matmul, fused
        # via one PSUM+SBUF add per feature tile
        kt1 = single.tile([128, SEQ], BF16, tag="kt1")
        kt2 = single.tile([128, SEQ], BF16, tag="kt2")
        kt3 = single.tile([128, SEQ], BF16, tag="kt3")
        for ft in range(NT):
            ps = ps_mm.tile([128, 2, 512], FP32, tag="mm")
            for nh in range(2):
                nc.tensor.matmul(ps[:, nh, :], wkvn_b[:, ft * 128:(ft + 1) * 128],
                                 ckvn[:, nh * 512:(nh + 1) * 512],
                                 start=True, stop=True)
            nc.vector.tensor_mul(kt1, kpe_rep, ck_b[:, ft, :])
            nc.vector.tensor_mul(kt2, kpe2_rep, sk_b[:, ft, :])
            nc.vector.tensor_add(kt3, kt1, kt2)
            nc.vector.tensor_add(kcat[ft], ps[:, :, :], kt3)

        # v (seq-major, augmented)
        for st in range(NT):
            ps = ps_mm.tile([128, 2, 512], FP32, tag="mm")
            for nh2 in range(2):
                nc.tensor.matmul(ps[:, nh2, :], ckvn[:, st * 128:(st + 1) * 128],
                                 wkvv_b[:, nh2 * 512:(nh2 + 1) * 512],
                                 start=True, stop=True)
            nc.vector.tensor_copy(
                vbuf[st][:, :, 0:64],
                ps[:, :, :].rearrange("p a (h v) -> p (a h) v", v=VD))

        # ---------- stage B: q path
        qln = single.tile([128, 2, QL], FP32, tag="qln")
        for mt2 in range(2):
            ps = ps_sc.tile([128, QL], FP32, tag="sc")
            for kt in range(NT):
                nc.tensor.matmul(ps, xRT_b[:, kt, mt2 * 128:(mt2 + 1) * 128],
                                 wqa_b[:, kt, :],
                                 start=(kt == 0), stop=(kt == NT - 1))
            sqq = work.tile([128, QL], FP32, tag="sqq")
            nc.scalar.activation(sqq, ps, AF.Square)
            ssum = work.tile([128, 1], FP32, tag="ssum")
            nc.vector.tensor_reduce(ssum, sqq, mybir.AxisListType.X, ALU.add)
            sdq = work.tile([128, 1], FP32, tag="sdq")
            nc.scalar.activation(sdq, ssum, AF.Sqrt, bias=eps128, scale=1.0 / QL)
            rq = work.tile([128, 1], FP32, tag="rq")
            nc.vector.reciprocal(rq, sdq)
            nc.vector.tensor_scalar_mul(qln[:, mt2, :], ps, rq)

        # transpose qln -> qlnT [20, 256] bf16
        qlnT = single.tile([QL, SB], BF16, tag="qlnT")
        for mt2 in range(2):
            tps = ps_sc.tile([QL, 128], FP32, tag="sc")
            nc.tensor.transpose(tps, qln[:, mt2, :], ident)
            nc.vector.tensor_copy(qlnT[:, mt2 * 128:(mt2 + 1) * 128], tps)

        # qT / q2 tiles (q2 weights padded to qcat layout)
        for mt in range(NT):
            ps = ps_sc.tile([128, SB], FP32, tag="sc")
            nc.tensor.matmul(ps, wqc_b[:, mt * 128:(mt + 1) * 128], qlnT,
                             start=True, stop=True)
            ps2 = ps_sc.tile([128, SB], FP32, tag="sc")
            nc.tensor.matmul(ps2, wqc_b[:, 1024 + mt * 128:1024 + (mt + 1) * 128],
                             qlnT, start=True, stop=True)
            qt1 = work.tile([128, SB], BF16, tag="qt1")
            qt2 = work.tile([128, SB], BF16, tag="qt2")
            nc.vector.tensor_mul(qt1, ps, cq_b[:, mt, :])          # nope: ps*1
            nc.vector.tensor_mul(qt2, ps2, sq_b[:, mt, :])         # nope: 0
            nc.vector.tensor_add(qcat[mt], qt1, qt2)

        # ---------- stage C: per-head scores / softmax / attn (unnormalized)
        dn_all = single.tile([16, SB], FP32, tag="dn_all")
        for h in range(NH):
            ft, po = h // 2, (h % 2) * 64
            ksl = lambda kt: kcat[ft][po:po + 64, kt * 128:(kt + 1) * 128]
            qsl = qcat[ft][po:po + 64, :]
            pt = ppool.tile([128, NT, SB], BF16, tag="pt")
            for kp in range(NT // 4):
                scps = ps_mm.tile([128, 2, 2, SB], FP32, tag="mm")
                for j in range(4):
                    nc.tensor.matmul(scps[:, j // 2, j % 2, :], ksl(4 * kp + j), qsl,
                                     start=True, stop=True)
                nc.scalar.activation(pt[:, 4 * kp:4 * kp + 4, :], scps, AF.Exp)
            atps = ps_att.tile([65, SB], FP32, tag="att")
            for kt in range(NT):
                nc.tensor.matmul(atps, vbuf[kt][:, h, :], pt[:, kt, :],
                                 start=(kt == 0), stop=(kt == NT - 1))
            nc.vector.tensor_copy(attn_b[ft][po:po + 64, :], atps[0:64, :])
            dtmp = work.tile([1, SB], FP32, tag="dtmp")
            nc.vector.tensor_copy(dtmp, atps[64:65, :])
            nc.gpsimd.dma_start(out=dn_all[h:h + 1, :], in_=dtmp)

        # batched denominator reciprocal + per-tile broadcast via select matmul
        rcp = single.tile([16, SB], FP32, tag="rcp")
        nc.vector.reciprocal(rcp, dn_all)

        # ---------- stage D: normalize + y^T = wo^T @ attn^T ; router
        for ft in range(NT):
            rps = ps_sc.tile([128, SB], FP32, tag="sc")
            nc.tensor.matmul(rps, sel_b[:, ft, :], rcp, start=True, stop=True)
            rb_sb = work.tile([128, SB], FP32, tag="rb_sb")
            nc.vector.tensor_copy(rb_sb, rps)
            nc.vector.tensor_mul(attn_n[ft], attn_b[ft], rb_sb)

        for mt in range(NT):
            ps = ps_sc.tile([128, SB], FP32, tag="sc")
            for ft in range(NT):
                nc.tensor.matmul(ps, wo_b[:, ft, mt * 128:(mt + 1) * 128],
                                 attn_n[ft],
                                 start=(ft == 0), stop=(ft == NT - 1))
            nc.vector.tensor_copy(yT_sb[mt], ps)
            nc.sync.dma_start(out=yT_out[mt * 128:(mt + 1) * 128, :],
                              in_=yT_sb[mt])

        rps = ps_sc.tile([NEXP, SB], FP32, tag="sc")
        for mt in range(NT):
            nc.tensor.matmul(rps, gw_b[:, mt, :], yT_sb[mt],
                             start=(mt == 0), stop=(mt == NT - 1))
        lg = single.tile([NEXP, SB], FP32, tag="lg")
        nc.vector.tensor_copy(lg, rps)
        for half in range(2):
            tps = ps_sc.tile([128, NEXP], FP32, tag="sc")
            nc.tensor.transpose(tps, lg[:, half * 128:(half + 1) * 128],
                                ident[0:NEXP, 0:NEXP])
            m = work.tile([128, 1], FP32, tag="m")
            nc.vector.tensor_reduce(m, tps, mybir.AxisListType.X, ALU.max)
            nm = work.tile([128, 1], FP32, tag="nm")
            nc.vector.tensor_scalar_mul(nm, m, -1.0)
            e = work.tile([128, NEXP], FP32, tag="e")
            nc.scalar.activation(e, tps, AF.Exp, bias=nm)
            s = work.tile([128, 1], FP32, tag="s")
            nc.vector.tensor_reduce(s, e, mybir.AxisListType.X, ALU.add)
            tw = work.tile([128, 1], FP32, tag="tw")
            nc.vector.reciprocal(tw, s)
            oh = work.tile([128, NEXP], FP32, tag="oh")
            nc.vector.tensor_scalar(oh, tps, m, None, ALU.is_equal)
            cmb = work.tile([128, NEXP], FP32, tag="cmb")
            nc.vector.tensor_scalar_mul(cmb, oh, tw)
            nc.sync.dma_start(out=comb_out[half * 128:(half + 1) * 128, :], in_=cmb)

    split_multi_waits(nc)
    return nc


# ================================================================= LAUNCH 2
def build_moe(TOK):
    nc = bass.Bass()
    xeT = nc.dram_tensor("xeT", [D, TOK], BF16, kind="ExternalInput")
    wrow = nc.dram_tensor("wrow", [1, TOK], FP32, kind="ExternalInput")
    wg = nc.dram_tensor("wg", [D, HID], BF16, kind="ExternalInput")
    wu = nc.dram_tensor("wu", [D, HID], BF16, kind="ExternalInput")
    wd = nc.dram_tensor("wd", [HID, D], BF16, kind="ExternalInput")
    outT = nc.dram_tensor("outT", [D, TOK], FP32, kind="ExternalOutput")

    chunks = []
    c0 = 0
    while c0 < TOK:
        cw = min(512, TOK - c0)
        chunks.append((c0, cw))
        c0 += cw

    with tile.TileContext(nc) as tc, contextlib.ExitStack() as ctx:
        const = ctx.enter_context(tc.tile_pool(name="const", bufs=1))
        work = ctx.enter_context(tc.tile_pool(name="work", bufs=3))
        ps_g = ctx.enter_context(tc.tile_pool(name="ps_g", bufs=2, space="PSUM"))
        ps_u = ctx.enter_context(tc.tile_pool(name="ps_u", bufs=2, space="PSUM"))
        ps_o = ctx.enter_context(tc.tile_pool(name="ps_o", bufs=2, space="PSUM"))

        def load_tiled(dram, cols, dt=BF16):
            ts = []
            for i in range(NT):
                t = const.tile([128, cols], dt, tag=f"ld_{dram.name}{i}")
                nc.sync.dma_start(out=t, in_=dram[i * 128:(i + 1) * 128, :])
                ts.append(t)
            return ts

        xe_b = load_tiled(xeT, TOK)
        wg_b = load_tiled(wg, HID)
        wu_b = load_tiled(wu, HID)
        wd_b = load_tiled(wd, D)
        wr_sb = const.tile([1, TOK], FP32)
        nc.sync.dma_start(out=wr_sb, in_=wrow[:, :])
        ones1 = const.tile([1, 128], FP32)
        nc.vector.memset(ones1, 1.0)
        wb = const.tile([128, TOK], FP32)
        bcast_row(nc, ps_o, ones1, wb, wr_sb[0:1, :], tag="bc")

        h_b = [const.tile([128, TOK], BF16, tag=f"h{i}") for i in range(NT)]

        for mt in range(NT):
            for (c0, cw) in chunks:
                cs = slice(c0, c0 + cw)
                gps = ps_g.tile([128, 512], FP32, tag="g")
                ups = ps_u.tile([128, 512], FP32, tag="u")
                for kt in range(NT):
                    nc.tensor.matmul(gps[:, :cw], wg_b[kt][:, mt * 128:(mt + 1) * 128],
                                     xe_b[kt][:, cs],
                                     start=(kt == 0), stop=(kt == NT - 1))
                for kt in range(NT):
                    nc.tensor.matmul(ups[:, :cw], wu_b[kt][:, mt * 128:(mt + 1) * 128],
                                     xe_b[kt][:, cs],
                                     start=(kt == 0), stop=(kt == NT - 1))
                sg = work.tile([128, 512], BF16, tag="sg")
                nc.scalar.activation(sg[:, :cw], gps[:, :cw], AF.Silu)
                if mt < 4:   # shared-expert hidden columns
                    nc.vector.tensor_mul(h_b[mt][:, cs], sg[:, :cw], ups[:, :cw])
                else:        # routed: also scale by per-token comb weight
                    hm = work.tile([128, 512], BF16, tag="hm")
                    nc.vector.tensor_mul(hm[:, :cw], sg[:, :cw], ups[:, :cw])
                    nc.vector.tensor_mul(h_b[mt][:, cs], hm[:, :cw], wb[:, cs])

        for dt_i in range(NT):
            for (c0, cw) in chunks:
                cs = slice(c0, c0 + cw)
                ops = ps_o.tile([128, 512], FP32, tag="o")
                for mt in range(NT):
                    nc.tensor.matmul(ops[:, :cw], wd_b[mt][:, dt_i * 128:(dt_i + 1) * 128],
                                     h_b[mt][:, cs],
                                     start=(mt == 0), stop=(mt == NT - 1))
                o_sb = work.tile([128, 512], FP32, tag="osb")
                nc.vector.tensor_copy(o_sb[:, :cw], ops[:, :cw])
                nc.sync.dma_start(out=outT[dt_i * 128:(dt_i + 1) * 128, cs],
                                  in_=o_sb[:, :cw])
    split_multi_waits(nc)
    return nc


# ================================================================ host glue
def _rope_tables():
    dim = ROPE * NH
    inv = 1.0 / (10000.0 ** (np.arange(0, dim, 2, dtype=np.float64) / dim))
    ang = (np.arange(SEQ, dtype=np.float64)[:, None] * inv[None, :])
    ang = ang.reshape(SEQ, NH, ROPE // 2).astype(np.float32)
    C = np.repeat(np.cos(ang), 2, axis=-1)   # [SEQ, NH, 32]
    S = np.repeat(np.sin(ang), 2, axis=-1)
    return C, S


def _swap_cols(W):
    W2 = np.empty_like(W)
    W2[..., 0::2] = -W[..., 1::2]
    W2[..., 1::2] = W[..., 0::2]
    return W2


def prep_attn_inputs(inputs):
    """Returns list of 8 in_maps (core = bi*4 + sb)."""
    f32 = np.float32
    x = np.asarray(inputs['dec_inp'], f32)
    wq_b = np.asarray(inputs['wq_b'], f32) * np.asarray(inputs['q_norm_w'], f32)[:, None]
    wkv_b = np.asarray(inputs['wkv_b'], f32) * np.asarray(inputs['kv_norm_w'], f32)[:, None]
    wkv_a = np.asarray(inputs['wkv_a'], f32)
    scale = 1.0 / np.sqrt(NOPE + ROPE)

    wq_b_s = (wq_b * scale).reshape(QL, NH, 64)
    wq_pe_sw = np.zeros((QL, NH, 64), np.float32)
    wq_pe_sw[:, :, NOPE:] = _swap_cols(wq_b_s[:, :, NOPE:])
    wq_cat = np.concatenate([wq_b_s.reshape(QL, NH * 64),
                             wq_pe_sw.reshape(QL, NH * 64)], axis=1)
    wkv_cat = np.concatenate([wkv_a[:, :QL], np.zeros((D, 12), f32),
                              wkv_a[:, QL:], _swap_cols(wkv_a[:, QL:])], axis=1)
    wkv_b_r = wkv_b.reshape(QL, NH, NOPE + VD)
    wkv_nope = np.zeros((QL, NH, 64), np.float32)
    wkv_nope[:, :, :NOPE] = wkv_b_r[:, :, :NOPE]
    wkv_nope = wkv_nope.reshape(QL, NH * 64)
    wkv_v = np.ascontiguousarray(wkv_b_r[:, :, NOPE:].reshape(QL, NH * VD))

    C, S = _rope_tables()
    ckf = np.zeros((128, NT, SEQ), np.float32)
    skf = np.zeros((128, NT, SEQ), np.float32)
    for h in range(NH):
        ft, po = h // 2, (h % 2) * 64
        ckf[po + 32:po + 64, ft, :] = C[:, h, :].T
        skf[po + 32:po + 64, ft, :] = S[:, h, :].T
    cqf = ckf.copy()
    for ft in range(NT):            # q combine: nope rows pass through (x1)
        cqf[0:32, ft, :] = 1.0
        cqf[64:96, ft, :] = 1.0
    self_sel = np.zeros((16, NT, 128), np.float32)
    for ft in range(NT):
        self_sel[2 * ft, ft, 0:64] = 1.0
        self_sel[2 * ft + 1, ft, 64:128] = 1.0

    shared = dict(
        wkv_cat=wkv_cat.astype(nbf),
        wq_a=np.asarray(inputs['wq_a'], f32).astype(nbf),
        wq_cat=wq_cat.astype(nbf),
        wkv_nope=wkv_nope.astype(nbf),
        wkv_v=wkv_v.astype(nbf),
        wo=np.asarray(inputs['wo'], f32).astype(nbf),
        gate_wT=np.ascontiguousarray(np.asarray(inputs['gate_w'], f32).T),
        ck=ckf.reshape(128, NT * SEQ).astype(nbf),
        sk=skf.reshape(128, NT * SEQ).astype(nbf),
        sel=self_sel.reshape(16, NT * 128),
        rep4=np.tile(np.eye(32, dtype=np.float32), (1, 4)).astype(nbf),
    )
    maps = []
    for bi in range(BATCH):
        xT = np.ascontiguousarray(x[:, bi, :].T).astype(nbf)
        for sb_i in range(4):
            R = slice(sb_i * SB, (sb_i + 1) * SB)
            m = dict(shared)
            m['xT'] = xT
            m['xRT'] = np.ascontiguousarray(xT[:, R])
            m['cq'] = np.ascontiguousarray(cqf[:, :, R]).reshape(128, NT * SB).astype(nbf)
            m['sq'] = np.ascontiguousarray(skf[:, :, R]).reshape(128, NT * SB).astype(nbf)
            maps.append(m)
    return maps


def assemble_attn(results):
    """results: list of 8 out dicts -> y [T, D] f32, comb [T, NEXP]."""
    T = SEQ * BATCH
    y = np.zeros((T, D), np.float32)
    comb = np.zeros((T, NEXP), np.float32)
    for core, res in enumerate(results):
        bi, sb_i = core // 4, core % 4
        tok = np.arange(sb_i * SB, (sb_i + 1) * SB) * BATCH + bi
        y[tok] = res['yT_out'].T
        comb[tok] = res['comb_out']
    return y, comb


def plan_moe(comb):
    assign = comb.argmax(axis=1)
    w = comb.max(axis=1)
    counts = [int((assign == e).sum()) for e in range(NEXP)]
    active = [e for e in range(NEXP) if counts[e] > 0]
    pairs = {e: 1 for e in active}
    while sum(pairs.values()) < 4:
        e = max(active, key=lambda e: counts[e] / pairs[e])
        pairs[e] += 1
    TOK = 128
    for e in active:
        TOK = max(TOK, -(-counts[e] // pairs[e]))
    TOK = -(-TOK // 128) * 128
    chunk_expert, chunk_toks = [], []
    for e in active:
        toks = np.where(assign == e)[0]
        for j in range(pairs[e]):
            chunk_expert.append(e)
            chunk_toks.append(toks[j * TOK:(j + 1) * TOK])
    return TOK, chunk_expert, chunk_toks, w, counts


def prep_moe_inputs(y, w, TOK, chunk_expert, chunk_toks, inputs):
    f32 = np.float32
    sh_g = np.asarray(inputs['sh_gate'], f32)
    sh_u = np.asarray(inputs['sh_up'], f32)
    sh_d = np.asarray(inputs['sh_down'], f32)
    eg = np.asarray(inputs['exp_gate'], f32)
    eu = np.asarray(inputs['exp_up'], f32)
    ed = np.asarray(inputs['exp_down'], f32)
    maps = []
    for j, (e, toks) in enumerate(zip(chunk_expert, chunk_toks)):
        xe = np.zeros((TOK, D), f32)
        xe[:len(toks)] = y[toks]
        xeT = np.ascontiguousarray(xe.T).astype(nbf)
        wr = np.zeros((1, TOK), f32)
        wr[0, :len(toks)] = w[toks]
        for hh in range(2):
            hs = slice(hh * 512, (hh + 1) * 512)
            maps.append(dict(
                xeT=xeT, wrow=wr,
                wg=np.concatenate([sh_g[:, hs], eg[e][:, hs]], axis=1).astype(nbf),
                wu=np.concatenate([sh_u[:, hs], eu[e][:, hs]], axis=1).astype(nbf),
                wd=np.concatenate([sh_d[hs], ed[e][hs]], axis=0).astype(nbf),
            ))
    return maps


def assemble_moe(results, chunk_toks):
    T = SEQ * BATCH
    out = np.zeros((T, D), np.float32)
    for j, toks in enumerate(chunk_toks):
        acc = results[2 * j]['outT'] + results[2 * j + 1]['outT']
        out[toks] = acc.T[:len(toks)]
    return out.reshape(SEQ, BATCH, D)


# ------------------------------------------------------------------ runner
_exec_cache = {}


def run_spmd_cached(key, nc, in_maps, trace=False, tmpdir=None):
    """Run via cached jitted executable (avoids NEFF recompile per call)."""
    from concourse.bass_utils import run_bass_kernel_spmd
    if trace:
        return run_bass_kernel_spmd(nc, in_maps, list(range(len(in_maps))),
                                    trace=True, tmpdir=tmpdir)
    import jax
    from jax.sharding import Mesh, PartitionSpec
    from jax.experimental.shard_map import shard_map
    from concourse import bass2jax

    n_cores = len(in_maps)
    if key not in _exec_cache:
        bass2jax.install_neuronx_cc_hook()
        in_names, out_names, out_avals, zero_outs = [], [], [], []
        for alloc in nc.m.functions[0].allocations:
            if not isinstance(alloc, mybir.MemoryLocationSet):
                continue
            name = alloc.memorylocations[0].name
            if alloc.kind == "ExternalInput":
                if not (nc.partition_id_tensor and name == nc.partition_id_tensor.name):
                    in_names.append(name)
            elif alloc.kind == "ExternalOutput":
                out_names.append(name)
                shape = tuple(alloc.tensor_shape)
                dtype = mybir.dt.np(alloc.dtype)
                out_avals.append(jax.core.ShapedArray(shape, dtype))
                zero_outs.append(np.zeros(shape, dtype))
        n_params = len(in_names)
        all_names = in_names + out_names
        if nc.partition_id_tensor is not None:
            all_names.append(nc.partition_id_tensor.name)

        def _body(*args):
            operands = list(args)
            if nc.partition_id_tensor is not None:
                operands.append(bass2jax.partition_id_tensor())
            outs = bass2jax._bass_exec_p.bind(
                *operands,
                out_avals=tuple(out_avals),
                in_names=tuple(all_names),
                out_names=tuple(out_names),
                lowering_input_output_aliases=(),
                sim_require_finite=True,
                sim_require_nnan=True,
                nc=nc,
            )
            return tuple(outs)

        devices = jax.devices()[:n_cores]
        mesh = Mesh(np.asarray(devices), ("core",))
        donate = tuple(range(n_params, n_params + len(out_names)))
        sharded = jax.jit(
            shard_map(_body, mesh=mesh,
                      in_specs=(PartitionSpec("core"),) * (n_params + len(out_names)),
                      out_specs=(PartitionSpec("core"),) * len(out_names),
                      check_rep=False),
            donate_argnums=donate, keep_unused=True)
        _exec_cache[key] = (sharded, in_names, out_names, zero_outs)

    sharded, in_names, out_names, zero_outs = _exec_cache[key]
    per_core = [[np.asarray(m[name]) for name in in_names] for m in in_maps]
    concat_in = [np.concatenate([per_core[c][i] for c in range(n_cores)], axis=0)
                 for i in range(len(in_names))]
    concat_zero = [np.concatenate([z] * n_cores, axis=0) for z in zero_outs]
    outs = sharded(*concat_in, *concat_zero)
    results = []
    for c in range(n_cores):
        res = {}
        for i, name in enumerate(out_names):
            full = np.asarray(outs[i])
            per = full.shape[0] // n_cores
            res[name] = full[c * per:(c + 1) * per]
        results.append(res)

    class R:
        pass
    r = R()
    r.results = results
    r.exec_time_ns = None
    return r


_nc_cache = {}


def kernel_impl(inputs, trace=False, tmpdir=None):
    if 'attn' not in _nc_cache:
        _nc_cache['attn'] = build_attn()
    nc_a = _nc_cache['attn']
    maps_a = prep_attn_inputs(inputs)
    ra = run_spmd_cached('attn', nc_a, maps_a, trace=trace,
                         tmpdir=(tmpdir + '/attn') if tmpdir else None)
    y, comb = assemble_attn(ra.results)

    TOK, chunk_expert, chunk_toks, w, counts = plan_moe(comb)
    mkey = ('moe', TOK)
    if mkey not in _nc_cache:
        _nc_cache[mkey] = build_moe(TOK)
    nc_m = _nc_cache[mkey]
    maps_m = prep_moe_inputs(y, w, TOK, chunk_expert, chunk_toks, inputs)
    rm = run_spmd_cached(mkey, nc_m, maps_m, trace=trace,
                         tmpdir=(tmpdir + '/moe') if tmpdir else None)
    out = assemble_moe(rm.results, chunk_toks)
    info = dict(counts=counts, TOK=TOK,
                t_attn=ra.exec_time_ns, t_moe=rm.exec_time_ns)
    return out, info


def kernel(**inputs):
    import numpy as np
    out, _info = kernel_impl({k: np.asarray(v) for k, v in inputs.items()})
    return out.astype(np.float32)
